# revision 20
# baseline (speedup 1.0000x reference)
"""DifferentialMultiHeadAttention TRN2 Bass kernel.

Sharding: 2 branches x 16 heads = 32 head-instances, 4 per core (core 0-3:
branch 1, core 4-7: branch 2). Each core computes its heads' attention,
applies its lambda-scaled head-output projection and the full final proj on
its rank-partial y; the host sums the 8 partial outputs (valid because wo,
the lambda-mix and proj are linear) and adds the folded bias vector.

Bias handling: q is augmented with the bias row (x' = [x, 1, 0..],
D 1024 -> 1152 = 9*128) so q = x@wq + bq exactly. k's bias is dropped on
device: (q+bq)@(k+bk) differs from (q+bq)@k by a per-query constant, which
cancels exactly in softmax. v's bias is moved to the host bias vector:
out = sum_s p_s (v+bv) = AV + bv since sum_s p_s = 1, so bv@wo'@proj is
added to the folded bias. This trims k/v to 8 contraction tiles (vs 9).

The final projection proj_w is folded into the per-head output projections
on the host (z = sum_h out_h @ (lamf * wo_h @ proj_w)), eliminating the
whole proj stage on device; each core emits a transposed partial zT in
bf16 (halves the output DMA; host sums in f64).

Softmax is computed without max-subtraction (scores are O(6), exp is safe in
fp32) via a transposed layout: scoresT[s,t] tiles feed exp (ScalarE,
PSUM->SBUF), then AV accumulates lhsT=[v|1] so PSUM row 64 is the softmax
denominator; the reciprocal row is broadcast across partitions with a K=1
outer-product matmul and applied with one vector multiply.

Pipelining: phase B (attention) of batch b drains a closure queue holding
(a) the previous chunk's head-output projections and (b) phase A (QKV) of
batch b+1 (kv tiles double-buffered), filling the PE bubbles left by the
exp dependency chain and removing the inter-batch phase gap.
"""

import sys

for _p in ("/opt/trn_rl_repo", "/opt/pypackages"):
    if _p not in sys.path:
        sys.path.append(_p)

import numpy as np
import ml_dtypes


MM_DTYPE = "bf16"   # "bf16" | "f32r"  (matmul operand precision)
MM_NP = ml_dtypes.bfloat16 if MM_DTYPE == "bf16" else np.float32

DIM, H, HD = 1024, 16, 64
B = 2
DA = 1152          # augmented contraction dim for q (bias row + pad)
NDT = DA // 128    # 9 d-tiles for q
NKV = DIM // 128   # 8 d-tiles for k/v (no bias row)
CH = 512           # token chunk size
NH = 4             # heads per core
NCORES = 8
NWARM = 40         # PE warmup matmuls covering initial weight/x DMA


def build(S=2048):
    """Build the per-core SPMD Bass program for per-batch seq len S."""
    import concourse.bacc as bacc
    import concourse.bass as bass
    import concourse.mybir as mybir
    import concourse.tile as tile

    f32 = mybir.dt.float32
    f32r = {"bf16": mybir.dt.bfloat16, "f32r": mybir.dt.float32r}[MM_DTYPE]

    T = B * S                    # total tokens
    NC = S // CH                 # chunks per batch
    NST = S // 128               # s-tiles per batch

    nc = bacc.Bacc("TRN2", target_bir_lowering=False, debug=False,
                   num_devices=NCORES)

    xta = nc.dram_tensor("xta", [DA, T], f32r, kind="ExternalInput")
    # weights arrive pre-transposed p-major from prep() so each DMA reads
    # long contiguous per-partition runs (4.6KB) instead of 512B strides
    wq = nc.dram_tensor("wq", [128, NDT * 256], f32r, kind="ExternalInput")
    wk = nc.dram_tensor("wk", [128, NKV * 256], f32r, kind="ExternalInput")
    wv = nc.dram_tensor("wv", [128, NKV * 256], f32r, kind="ExternalInput")
    wo = nc.dram_tensor("wo", [128, 2 * DIM], f32r, kind="ExternalInput")
    z = nc.dram_tensor("z", [DIM, T], f32r, kind="ExternalOutput")

    with tile.TileContext(nc) as tc:
        with (
            nc.allow_low_precision(
                reason="bf16 operands; PSUM accumulation stays fp32"),
            tc.tile_pool(name="consts", bufs=1) as consts,
            tc.tile_pool(name="kv", bufs=2) as kv,
            tc.tile_pool(name="xp", bufs=2) as xp,
            tc.tile_pool(name="work", bufs=4) as work,
            tc.tile_pool(name="outp", bufs=2) as outp,
            tc.tile_pool(name="scp", bufs=2, space="PSUM") as scp,
            tc.tile_pool(name="flx", bufs=4, space="PSUM") as flx,
        ):
            wq_sb = consts.tile([128, NDT, 256], f32r)
            wk_sb = consts.tile([128, NKV, 256], f32r)
            wv_sb = consts.tile([128, NKV, 256], f32r)
            wo_sb = consts.tile([128, 2, DIM], f32r)

            # wq/wk on the fast sync queue ahead of everything else: the
            # first q/k matmuls gate on them plus x chunk 0
            nc.sync.dma_start(out=wq_sb, in_=wq.ap().rearrange("p (dt m) -> p dt m", m=256))
            nc.sync.dma_start(out=wk_sb, in_=wk.ap().rearrange("p (dt m) -> p dt m", m=256))
            nc.gpsimd.dma_start(out=wv_sb, in_=wv.ap().rearrange("p (dt m) -> p dt m", m=256))
            # wo' (= lamf * wo @ proj_w, folded host-side) is first used in
            # phase B; keep it off the startup critical path
            nc.gpsimd.dma_start(out=wo_sb, in_=wo.ap().rearrange("p (pk n) -> p pk n", n=DIM))

            xre = xta.ap().rearrange("(dt p) t -> p dt t", p=128)

            wu = consts.tile([128, CH], f32r, name="wu")
            nc.vector.memset(wu, 0.25)
            for wi in range(NWARM):
                wp = flx.tile([128, CH], f32, tag="flex", name=f"wp{wi}")
                nc.tensor.matmul(wp[:], wu[:, 0:128], wu[:], start=True, stop=True)

            opq = []

            def drain(n):
                for _ in range(min(n, len(opq))):
                    opq.pop(0)()

            def queue_zt(tb, outT):
                # head-output projection (wo' = lamf * wo @ proj_w folded on
                # the host) of a finished chunk, split into closures drained
                # one at a time during later attention sections to fill PE
                # bubbles. Output is zT [DIM, T] bf16; host sums+transposes.
                def zt_op(eo):
                    def f():
                        zp = flx.tile([128, CH], f32, tag="flex",
                                      name=f"zp{tb}_{eo}")
                        for pk in range(2):
                            nc.tensor.matmul(
                                zp[:], (wo_sb[:, pk, eo * 128:(eo + 1) * 128]),
                                (outT[:, pk, :]),
                                start=(pk == 0), stop=(pk == 1))
                        zs = work.tile([128, CH], f32r, tag="zs",
                                       name=f"zs{tb}_{eo}")
                        nc.vector.tensor_copy(zs[:], zp[:])
                        nc.sync.dma_start(
                            out=z.ap()[eo * 128:(eo + 1) * 128, tb:tb + CH],
                            in_=zs[:])
                    return f

                for eo in range(NPT_):
                    opq.append(zt_op(eo))

            NPT_ = DIM // 128

            def alloc_kv(b):
                kT = kv.tile([128, 2, S], f32r, tag="kT", name=f"kT{b}")
                qT = kv.tile([128, 2, S], f32r, tag="qT", name=f"qT{b}")
                va = kv.tile([128, NST, NH, 65], f32r, tag="va",
                             name=f"va{b}")
                # ones column via memset (a strided scatter DMA here would
                # sit on the sync queue ahead of the x fetches)
                nc.vector.memset(va[:, :, :, 64:65], 1.0)
                return {"kT": kT, "qT": qT, "va": va}

            def fetch_x(b, c):
                # split across two DMA queues to double the fetch rate (the
                # rearranged AP is 1KB-run limited per queue)
                tb = b * S + c * CH
                x_blk = xp.tile([128, NDT, CH], f32r, tag="x",
                                name=f"x{b}_{c}")
                nc.sync.dma_start(out=x_blk[:, 0:5, :],
                                  in_=xre[:, 0:5, tb:tb + CH])
                nc.gpsimd.dma_start(out=x_blk[:, 5:NDT, :],
                                    in_=xre[:, 5:NDT, tb:tb + CH])
                return x_blk

            def qkv_ops(kvd, c, x_blk):
                # phase A work for one chunk as a list of small closures
                # (~0.5-1.1us of PE each, split so a drained closure never
                # stalls the attention score pipeline). Emitted directly for
                # batch 0; queued into opq for batch b+1 during batch b's
                # phase B.
                ops = []

                def qk_op(wsb, ndt, dst, pk):
                    cell = {}

                    def f0():
                        ps = flx.tile([128, CH], f32, tag="flex")
                        cell["ps"] = ps
                        for dt_i in range(ndt // 2):
                            nc.tensor.matmul(
                                ps[:], (wsb[:, dt_i, 128 * pk:128 * pk + 128]),
                                (x_blk[:, dt_i, :]),
                                start=(dt_i == 0), stop=False)

                    def f1():
                        ps = cell["ps"]
                        for dt_i in range(ndt // 2, ndt):
                            nc.tensor.matmul(
                                ps[:], (wsb[:, dt_i, 128 * pk:128 * pk + 128]),
                                (x_blk[:, dt_i, :]),
                                start=False, stop=(dt_i == ndt - 1))
                        nc.vector.tensor_copy(
                            dst[:, pk, c * CH:(c + 1) * CH], ps[:])
                    return [f0, f1]

                def v_op(tt):
                    cell = {}

                    def f0():
                        ps = flx.tile([128, 256], f32, tag="flex")
                        cell["ps"] = ps
                        for dt_i in range(NKV // 2):
                            nc.tensor.matmul(
                                ps[:], (x_blk[:, dt_i, 128 * tt:128 * tt + 128]),
                                (wv_sb[:, dt_i, :]),
                                start=(dt_i == 0), stop=False)

                    def f1():
                        ps = cell["ps"]
                        for dt_i in range(NKV // 2, NKV):
                            nc.tensor.matmul(
                                ps[:], (x_blk[:, dt_i, 128 * tt:128 * tt + 128]),
                                (wv_sb[:, dt_i, :]),
                                start=False, stop=(dt_i == NKV - 1))
                        st = c * 4 + tt
                        nc.vector.tensor_copy(
                            kvd["va"][:, st, :, 0:64],
                            ps.rearrange("p (h d) -> p h d", h=NH))
                    return [f0, f1]

                for wsb, ndt, dst in ((wq_sb, NDT, kvd["qT"]),
                                      (wk_sb, NKV, kvd["kT"])):
                    for pk in range(2):
                        ops.extend(qk_op(wsb, ndt, dst, pk))
                for tt in range(4):
                    ops.extend(v_op(tt))
                return ops

            kvts = [alloc_kv(0)]

            # ---- phase A for batch 0 (emitted directly) ----
            for c in range(NC):
                x_blk = fetch_x(0, c)
                for op in qkv_ops(kvts[0], c, x_blk):
                    op()

            for b in range(B):
                kvd = kvts[b]
                kT, qT, va = kvd["kT"], kvd["qT"], kvd["va"]
                if b + 1 < B:
                    kvts.append(alloc_kv(b + 1))
                    x_next = fetch_x(b + 1, 0)

                # ---- phase B: attention per chunk; zt of the previous
                # chunk and QKV of the next batch drain through opq to fill
                # PE bubbles ----
                for c in range(NC):
                    tb = b * S + c * CH
                    if b + 1 < B:
                        opq.extend(qkv_ops(kvts[b + 1], c, x_next))
                        if c + 1 < NC:
                            x_next = fetch_x(b + 1, c + 1)
                    outT = outp.tile([128, 2, CH], f32r, tag="outT",
                                     name=f"outT{b}_{c}")

                    for pk in range(2):
                        # head pair (2*pk, 2*pk+1): per s-tile ONE hh-major
                        # score tile [128, 2(hh), CH] -> ONE exp -> 2 AV
                        # accumulations. scp bufs=2 gives true double
                        # buffering: scores of s-tile st+1 overlap exp of st.
                        avs = [flx.tile([128, CH], f32, tag="flex",
                                        name=f"av{pk}_{i}")
                               for i in range(2)]
                        def av_pair(st, ex):
                            for hh in range(2):
                                h = 2 * pk + hh
                                nc.tensor.matmul(
                                    avs[hh][0:65, :], (va[:, st, h, :]),
                                    (ex[:, hh, :]),
                                    start=(st == 0), stop=(st == NST - 1))

                        # software-pipelined: AV lags scores by one s-tile so
                        # the PE streams scores(st+1) while exp(st) runs on
                        # ScalarE instead of head-of-line blocking on it.
                        # software-pipelined with AV lagging scores by TWO
                        # s-tiles: exp(st) has a full extra period to finish
                        # before AV(st) is issued, so AV never waits on the
                        # ScalarE sem.
                        pend = []
                        for st in range(NST):
                            sc = scp.tile([128, 2, CH], f32, tag="sc",
                                          name=f"sc{pk}_{st}")
                            for hh in range(2):
                                row = 64 * hh
                                nc.tensor.matmul(
                                    sc[:, hh, :],
                                    (kT[row:row + 64, pk, st * 128:(st + 1) * 128]),
                                    (qT[row:row + 64, pk, c * CH:(c + 1) * CH]),
                                    start=True, stop=True)
                            ex = work.tile([128, 2, CH], f32r, tag="ex",
                                           bufs=6)
                            nc.scalar.activation(
                                ex[:], sc[:],
                                mybir.ActivationFunctionType.Exp)
                            pend.append((st, ex))
                            if len(pend) > 2:
                                av_pair(*pend.pop(0))
                            drain(1)
                        # cover the trailing exps with queued work
                        while pend:
                            drain(1)
                            av_pair(*pend.pop(0))

                        dens, us = [], []
                        for hh in range(2):
                            # den + u copies emitted immediately to free both
                            # av PSUM slots; the reciprocal/broadcast tail is
                            # queued so the next section's scores aren't
                            # head-of-line blocked behind it on the PE queue.
                            av = avs[hh]
                            den = work.tile([1, CH], f32, tag="den",
                                            name=f"den{pk}_{hh}")
                            nc.vector.tensor_copy(den[:], av[64:65, :])
                            u = work.tile([64, CH], f32r, tag="u", bufs=6,
                                          name=f"u{pk}_{hh}")
                            nc.vector.tensor_copy(u[:], av[0:64, :])
                            dens.append(den); us.append(u)

                        def norm_op(pk, dens, us, outT):
                            def f():
                                # reciprocal row broadcast across partitions
                                # via the DMA path (partition_broadcast) --
                                # zero PE involvement in normalization
                                for hh in range(2):
                                    rcp = work.tile([1, CH], f32, tag="rcp",
                                                    name=f"rcp{pk}_{hh}")
                                    nc.vector.reciprocal_approx_fast(
                                        rcp[:], dens[hh][:])
                                    rcpm = work.tile([1, CH], f32r, tag="rcpm",
                                                     bufs=6,
                                                     name=f"rcpm{pk}_{hh}")
                                    nc.vector.tensor_copy(rcpm[:], rcp[:])
                                    bcb = work.tile([64, CH], f32r, tag="bcb",
                                                    bufs=4,
                                                    name=f"bcb{pk}_{hh}")
                                    nc.gpsimd.partition_broadcast(
                                        bcb[:], rcpm[:])
                                    nc.vector.tensor_mul(
                                        outT[64 * hh:64 * hh + 64, pk, :],
                                        us[hh][:], bcb[:])
                            return f

                        opq.append(norm_op(pk, dens, us, outT))

                    queue_zt(tb, outT)

            drain(len(opq))

    nc.compile()
    return nc


def get_lambda(lambda_param, layer_idx):
    lf = np.clip(float(np.asarray(layer_idx)) * 0.3, 0.0, 5.0)
    offset = 0.6 * np.exp(-lf)
    lam = (1.0 / (1.0 + np.exp(-float(np.asarray(lambda_param).reshape(-1)[0])))
           ) * (1.0 - offset) + 0.2
    return float(np.clip(lam, 0.1, 0.9))


def prep(inputs, S=2048):
    """Host-side shard prep: returns (in_maps, bias_vec)."""
    x = np.asarray(inputs["x"], np.float32)
    T = B * S
    x2 = np.ascontiguousarray(x.reshape(T, DIM))
    xta = np.zeros((DA, T), np.float32)
    xta[:DIM] = x2.T
    xta[DIM] = 1.0

    lam = get_lambda(inputs["lambda_param"], inputs["layer_idx"])
    pw = np.asarray(inputs["proj_w"], np.float32)
    xta_mm = xta.astype(MM_NP)

    in_maps = []
    for c in range(NCORES):
        br = c // 4 + 1
        lamf = (1.0 - lam) if br == 1 else lam
        hs = slice(4 * (c % 4), 4 * (c % 4) + 4)

        def pmajor(w2d):
            # [ndt*128, m] -> [128, ndt*m] so each partition's DMA run is
            # contiguous
            ndt = w2d.shape[0] // 128
            m = w2d.shape[1]
            return np.ascontiguousarray(
                w2d.reshape(ndt, 128, m).transpose(1, 0, 2).reshape(
                    128, ndt * m)).astype(MM_NP)

        def aug_q(w, bias, scale):
            wa = np.zeros((DA, NH, HD), np.float32)
            wa[:DIM] = np.asarray(w, np.float32)[:, hs]
            wa[DIM] = np.asarray(bias, np.float32)[hs]
            return pmajor((wa * scale).reshape(DA, NH * HD))

        def plain(w):
            wa = np.asarray(w, np.float32)[:, hs]
            return pmajor(wa.reshape(DIM, NH * HD))

        wo_c = pmajor(
            (np.asarray(inputs[f"wo{br}"], np.float32)[hs] * lamf
             ).reshape(256, DIM) @ pw)
        in_maps.append({
            "xta": xta_mm,
            "wq": aug_q(inputs[f"wq{br}"], inputs[f"bq{br}"], 1.0 / np.sqrt(HD)),
            "wk": plain(inputs[f"wk{br}"]),
            "wv": plain(inputs[f"wv{br}"]),
            "wo": wo_c,
        })

    lam64 = np.float64(np.float32(lam))
    yb = np.zeros((DIM,), np.float64)
    for br, lamf in ((1, 1.0 - lam64), (2, lam64)):
        # bo and the dropped v-bias, both through wo (exact: sum_s p_s = 1)
        wof = np.asarray(inputs[f"wo{br}"], np.float64).reshape(H * HD, DIM)
        yb += lamf * (np.asarray(inputs[f"bo{br}"], np.float64)
                      + np.asarray(inputs[f"bv{br}"], np.float64).reshape(H * HD)
                      @ wof)
    bias_vec = yb @ pw.astype(np.float64) \
        + np.asarray(inputs["proj_b"], np.float64)
    return in_maps, bias_vec


_NC_CACHE = {}


def _get_nc(S=2048):
    if S not in _NC_CACHE:
        _NC_CACHE[S] = build(S)
    return _NC_CACHE[S]


def run(inputs, S=2048, trace=False):
    """Returns (full_output, exec_time_ns_or_None)."""
    from concourse import bass_utils

    nc = _get_nc(S)
    in_maps, bias_vec = prep(inputs, S)
    res = bass_utils.run_bass_kernel_spmd(
        nc, in_maps, core_ids=list(range(NCORES)), trace=trace)
    accT = np.zeros((DIM, B * S), np.float64)
    for c in range(NCORES):
        accT += res.results[c]["z"].astype(np.float64)
    out = (accT.T + bias_vec).reshape(B, S, DIM).astype(np.float32)
    return out, res.exec_time_ns


def kernel(**inputs):
    out, _ = run(inputs, S=2048, trace=False)
    return out


# revision 22
# speedup vs baseline: 1.1064x; 1.1064x over previous
"""DifferentialMultiHeadAttention TRN2 Bass kernel.

Sharding: 2 branches x 16 heads = 32 head-instances, 4 per core (core 0-3:
branch 1, core 4-7: branch 2). Each core computes its heads' attention,
applies its lambda-scaled head-output projection and the full final proj on
its rank-partial y; the host sums the 8 partial outputs (valid because wo,
the lambda-mix and proj are linear) and adds the folded bias vector.

Bias handling: q is augmented with the bias row (x' = [x, 1, 0..],
D 1024 -> 1152 = 9*128) so q = x@wq + bq exactly. k's bias is dropped on
device: (q+bq)@(k+bk) differs from (q+bq)@k by a per-query constant, which
cancels exactly in softmax. v's bias is moved to the host bias vector:
out = sum_s p_s (v+bv) = AV + bv since sum_s p_s = 1, so bv@wo'@proj is
added to the folded bias. This trims k/v to 8 contraction tiles (vs 9).

The final projection proj_w is folded into the per-head output projections
on the host (z = sum_h out_h @ (lamf * wo_h @ proj_w)), eliminating the
whole proj stage on device; each core emits a transposed partial zT in
bf16 (halves the output DMA; host sums in f64).

Softmax is computed without max-subtraction (scores are O(6), exp is safe in
fp32) via a transposed layout: scoresT[s,t] tiles feed exp (ScalarE,
PSUM->SBUF), then AV accumulates lhsT=[v|1] so PSUM row 64 is the softmax
denominator; the reciprocal row is broadcast across partitions with a K=1
outer-product matmul and applied with one vector multiply.

Pipelining: phase B (attention) of batch b drains a closure queue holding
(a) the previous chunk's head-output projections and (b) phase A (QKV) of
batch b+1 (kv tiles double-buffered), filling the PE bubbles left by the
exp dependency chain and removing the inter-batch phase gap.
"""

import sys

for _p in ("/opt/trn_rl_repo", "/opt/pypackages"):
    if _p not in sys.path:
        sys.path.append(_p)

import numpy as np
import ml_dtypes


MM_DTYPE = "bf16"   # "bf16" | "f32r"  (matmul operand precision)
MM_NP = ml_dtypes.bfloat16 if MM_DTYPE == "bf16" else np.float32

DIM, H, HD = 1024, 16, 64
B = 2
DA = 1152          # augmented contraction dim for q (bias row + pad)
NDT = DA // 128    # 9 d-tiles for q
NKV = DIM // 128   # 8 d-tiles for k/v (no bias row)
CH = 512           # token chunk size
NH = 4             # heads per core
NCORES = 8
NWARM = 40         # PE warmup matmuls covering initial weight/x DMA


def build(S=2048):
    """Build the per-core SPMD Bass program for per-batch seq len S."""
    import concourse.bacc as bacc
    import concourse.bass as bass
    import concourse.mybir as mybir
    import concourse.tile as tile

    f32 = mybir.dt.float32
    f32r = {"bf16": mybir.dt.bfloat16, "f32r": mybir.dt.float32r}[MM_DTYPE]

    T = B * S                    # total tokens
    NC = S // CH                 # chunks per batch
    NST = S // 128               # s-tiles per batch

    nc = bacc.Bacc("TRN2", target_bir_lowering=False, debug=False,
                   num_devices=NCORES)

    xta = nc.dram_tensor("xta", [DA, T], f32r, kind="ExternalInput")
    # weights arrive pre-transposed p-major from prep() so each DMA reads
    # long contiguous per-partition runs (4.6KB) instead of 512B strides
    wq = nc.dram_tensor("wq", [128, NDT * 256], f32r, kind="ExternalInput")
    wk = nc.dram_tensor("wk", [128, NKV * 256], f32r, kind="ExternalInput")
    wv = nc.dram_tensor("wv", [128, NKV * 256], f32r, kind="ExternalInput")
    wo = nc.dram_tensor("wo", [128, 2 * DIM], f32r, kind="ExternalInput")
    z = nc.dram_tensor("z", [DIM, T], f32r, kind="ExternalOutput")

    with tile.TileContext(nc) as tc:
        with (
            nc.allow_low_precision(
                reason="bf16 operands; PSUM accumulation stays fp32"),
            tc.tile_pool(name="consts", bufs=1) as consts,
            tc.tile_pool(name="kv", bufs=2) as kv,
            tc.tile_pool(name="xp", bufs=2) as xp,
            tc.tile_pool(name="work", bufs=4) as work,
            tc.tile_pool(name="outp", bufs=2) as outp,
            tc.tile_pool(name="scp", bufs=2, space="PSUM") as scp,
            tc.tile_pool(name="flx", bufs=4, space="PSUM") as flx,
        ):
            wq_sb = consts.tile([128, NDT, 256], f32r)
            wk_sb = consts.tile([128, NKV, 256], f32r)
            wv_sb = consts.tile([128, NKV, 256], f32r)
            wo_sb = consts.tile([128, 2, DIM], f32r)

            # wq/wk on the fast sync queue ahead of everything else: the
            # first q/k matmuls gate on them plus x chunk 0
            nc.sync.dma_start(out=wq_sb, in_=wq.ap().rearrange("p (dt m) -> p dt m", m=256))
            nc.sync.dma_start(out=wk_sb, in_=wk.ap().rearrange("p (dt m) -> p dt m", m=256))
            nc.gpsimd.dma_start(out=wv_sb, in_=wv.ap().rearrange("p (dt m) -> p dt m", m=256))
            # wo' (= lamf * wo @ proj_w, folded host-side) is first used in
            # phase B; keep it off the startup critical path
            nc.gpsimd.dma_start(out=wo_sb, in_=wo.ap().rearrange("p (pk n) -> p pk n", n=DIM))

            xre = xta.ap().rearrange("(dt p) t -> p dt t", p=128)

            ones_mm = consts.tile([1, 64], f32r)
            nc.vector.memset(ones_mm, 1.0)

            wu = consts.tile([128, CH], f32r, name="wu")
            nc.vector.memset(wu, 0.25)
            for wi in range(NWARM):
                wp = flx.tile([128, CH], f32, tag="flex", name=f"wp{wi}")
                nc.tensor.matmul(wp[:], wu[:, 0:128], wu[:], start=True, stop=True)

            opq = []

            def drain(n):
                for _ in range(min(n, len(opq))):
                    opq.pop(0)()

            def queue_zt(tb, outT):
                # head-output projection (wo' = lamf * wo @ proj_w folded on
                # the host) of a finished chunk, split into closures drained
                # one at a time during later attention sections to fill PE
                # bubbles. Output is zT [DIM, T] bf16; host sums+transposes.
                def zt_op(eo):
                    def f():
                        zp = flx.tile([128, CH], f32, tag="flex",
                                      name=f"zp{tb}_{eo}")
                        for pk in range(2):
                            nc.tensor.matmul(
                                zp[:], (wo_sb[:, pk, eo * 128:(eo + 1) * 128]),
                                (outT[:, pk, :]),
                                start=(pk == 0), stop=(pk == 1))
                        zs = work.tile([128, CH], f32r, tag="zs",
                                       name=f"zs{tb}_{eo}")
                        nc.vector.tensor_copy(zs[:], zp[:])
                        nc.sync.dma_start(
                            out=z.ap()[eo * 128:(eo + 1) * 128, tb:tb + CH],
                            in_=zs[:])
                    return f

                for eo in range(NPT_):
                    opq.append(zt_op(eo))

            NPT_ = DIM // 128

            def alloc_kv(b):
                kT = kv.tile([128, 2, S], f32r, tag="kT", name=f"kT{b}")
                qT = kv.tile([128, 2, S], f32r, tag="qT", name=f"qT{b}")
                va = kv.tile([128, NST, NH, 65], f32r, tag="va",
                             name=f"va{b}")
                # ones column via memset (a strided scatter DMA here would
                # sit on the sync queue ahead of the x fetches)
                nc.vector.memset(va[:, :, :, 64:65], 1.0)
                return {"kT": kT, "qT": qT, "va": va}

            def fetch_x(b, c):
                # split across two DMA queues to double the fetch rate (the
                # rearranged AP is 1KB-run limited per queue)
                tb = b * S + c * CH
                x_blk = xp.tile([128, NDT, CH], f32r, tag="x",
                                name=f"x{b}_{c}")
                nc.sync.dma_start(out=x_blk[:, 0:5, :],
                                  in_=xre[:, 0:5, tb:tb + CH])
                nc.gpsimd.dma_start(out=x_blk[:, 5:NDT, :],
                                    in_=xre[:, 5:NDT, tb:tb + CH])
                return x_blk

            def qkv_ops(kvd, c, x_blk):
                # phase A work for one chunk as a list of small closures
                # (~0.5-1.1us of PE each, split so a drained closure never
                # stalls the attention score pipeline). Emitted directly for
                # batch 0; queued into opq for batch b+1 during batch b's
                # phase B.
                ops = []

                def qk_op(wsb, ndt, dst, pk):
                    cell = {}

                    def f0():
                        ps = flx.tile([128, CH], f32, tag="flex")
                        cell["ps"] = ps
                        for dt_i in range(ndt // 2):
                            nc.tensor.matmul(
                                ps[:], (wsb[:, dt_i, 128 * pk:128 * pk + 128]),
                                (x_blk[:, dt_i, :]),
                                start=(dt_i == 0), stop=False)

                    def f1():
                        ps = cell["ps"]
                        for dt_i in range(ndt // 2, ndt):
                            nc.tensor.matmul(
                                ps[:], (wsb[:, dt_i, 128 * pk:128 * pk + 128]),
                                (x_blk[:, dt_i, :]),
                                start=False, stop=(dt_i == ndt - 1))
                        nc.vector.tensor_copy(
                            dst[:, pk, c * CH:(c + 1) * CH], ps[:])
                    return [f0, f1]

                def v_op(tt):
                    cell = {}

                    def f0():
                        ps = flx.tile([128, 256], f32, tag="flex")
                        cell["ps"] = ps
                        for dt_i in range(NKV // 2):
                            nc.tensor.matmul(
                                ps[:], (x_blk[:, dt_i, 128 * tt:128 * tt + 128]),
                                (wv_sb[:, dt_i, :]),
                                start=(dt_i == 0), stop=False)

                    def f1():
                        ps = cell["ps"]
                        for dt_i in range(NKV // 2, NKV):
                            nc.tensor.matmul(
                                ps[:], (x_blk[:, dt_i, 128 * tt:128 * tt + 128]),
                                (wv_sb[:, dt_i, :]),
                                start=False, stop=(dt_i == NKV - 1))
                        st = c * 4 + tt
                        nc.vector.tensor_copy(
                            kvd["va"][:, st, :, 0:64],
                            ps.rearrange("p (h d) -> p h d", h=NH))
                    return [f0, f1]

                for wsb, ndt, dst in ((wq_sb, NDT, kvd["qT"]),
                                      (wk_sb, NKV, kvd["kT"])):
                    for pk in range(2):
                        ops.extend(qk_op(wsb, ndt, dst, pk))
                for tt in range(4):
                    ops.extend(v_op(tt))
                return ops

            kvts = [alloc_kv(0)]

            # ---- phase A for batch 0 (emitted directly) ----
            for c in range(NC):
                x_blk = fetch_x(0, c)
                for op in qkv_ops(kvts[0], c, x_blk):
                    op()

            for b in range(B):
                kvd = kvts[b]
                kT, qT, va = kvd["kT"], kvd["qT"], kvd["va"]
                if b + 1 < B:
                    kvts.append(alloc_kv(b + 1))
                    x_next = fetch_x(b + 1, 0)

                # ---- phase B: attention per chunk; zt of the previous
                # chunk and QKV of the next batch drain through opq to fill
                # PE bubbles ----
                for c in range(NC):
                    tb = b * S + c * CH
                    if b + 1 < B:
                        opq.extend(qkv_ops(kvts[b + 1], c, x_next))
                        if c + 1 < NC:
                            x_next = fetch_x(b + 1, c + 1)
                    outT = outp.tile([128, 2, CH], f32r, tag="outT",
                                     name=f"outT{b}_{c}")

                    for pk in range(2):
                        # head pair (2*pk, 2*pk+1): per s-tile ONE hh-major
                        # score tile [128, 2(hh), CH] -> ONE exp -> 2 AV
                        # accumulations. scp bufs=2 gives true double
                        # buffering: scores of s-tile st+1 overlap exp of st.
                        avs = [flx.tile([128, CH], f32, tag="flex",
                                        name=f"av{pk}_{i}")
                               for i in range(2)]
                        def av_pair(st, ex):
                            for hh in range(2):
                                h = 2 * pk + hh
                                nc.tensor.matmul(
                                    avs[hh][0:65, :], (va[:, st, h, :]),
                                    (ex[:, hh, :]),
                                    start=(st == 0), stop=(st == NST - 1))

                        # software-pipelined: AV lags scores by one s-tile so
                        # the PE streams scores(st+1) while exp(st) runs on
                        # ScalarE instead of head-of-line blocking on it.
                        # software-pipelined with AV lagging scores by TWO
                        # s-tiles: exp(st) has a full extra period to finish
                        # before AV(st) is issued, so AV never waits on the
                        # ScalarE sem.
                        pend = []
                        for st in range(NST):
                            sc = scp.tile([128, 2, CH], f32, tag="sc",
                                          name=f"sc{pk}_{st}")
                            for hh in range(2):
                                row = 64 * hh
                                nc.tensor.matmul(
                                    sc[:, hh, :],
                                    (kT[row:row + 64, pk, st * 128:(st + 1) * 128]),
                                    (qT[row:row + 64, pk, c * CH:(c + 1) * CH]),
                                    start=True, stop=True)
                            ex = work.tile([128, 2, CH], f32r, tag="ex",
                                           bufs=6)
                            nc.scalar.activation(
                                ex[:], sc[:],
                                mybir.ActivationFunctionType.Exp)
                            pend.append((st, ex))
                            if len(pend) > 2:
                                av_pair(*pend.pop(0))
                            drain(1)
                        # cover the trailing exps with queued work
                        while pend:
                            drain(1)
                            av_pair(*pend.pop(0))

                        dens, us = [], []
                        for hh in range(2):
                            # den + u copies emitted immediately to free both
                            # av PSUM slots; the reciprocal/broadcast tail is
                            # queued so the next section's scores aren't
                            # head-of-line blocked behind it on the PE queue.
                            av = avs[hh]
                            den = work.tile([1, CH], f32, tag="den",
                                            name=f"den{pk}_{hh}")
                            nc.vector.tensor_copy(den[:], av[64:65, :])
                            u = work.tile([64, CH], f32r, tag="u", bufs=6,
                                          name=f"u{pk}_{hh}")
                            nc.vector.tensor_copy(u[:], av[0:64, :])
                            dens.append(den); us.append(u)

                        # reciprocal chain emitted now (VectorE, off the PE
                        # path); the PE broadcast+mul is queued and inserted
                        # a few slots deep so its bc matmul never head-of-line
                        # blocks the next section's scores on a fresh chain.
                        rcpms = []
                        for hh in range(2):
                            rcp = work.tile([1, CH], f32, tag="rcp",
                                            name=f"rcp{pk}_{hh}")
                            nc.vector.reciprocal_approx_fast(
                                rcp[:], dens[hh][:])
                            rcpm = work.tile([1, CH], f32r, tag="rcpm",
                                             bufs=6, name=f"rcpm{pk}_{hh}")
                            nc.vector.tensor_copy(rcpm[:], rcp[:])
                            rcpms.append(rcpm)

                        def norm_op(pk, rcpms, us, outT):
                            def f():
                                for hh in range(2):
                                    bc = flx.tile([64, CH], f32, tag="flex",
                                                  name=f"bc{pk}_{hh}")
                                    nc.tensor.matmul(bc[:], (ones_mm[:]),
                                                     (rcpms[hh][:]),
                                                     start=True, stop=True)
                                    nc.vector.tensor_mul(
                                        outT[64 * hh:64 * hh + 64, pk, :],
                                        us[hh][:], bc[:])
                            return f

                        opq.insert(min(3, len(opq)),
                                   norm_op(pk, rcpms, us, outT))

                    queue_zt(tb, outT)

            drain(len(opq))

    nc.compile()
    return nc


def get_lambda(lambda_param, layer_idx):
    lf = np.clip(float(np.asarray(layer_idx)) * 0.3, 0.0, 5.0)
    offset = 0.6 * np.exp(-lf)
    lam = (1.0 / (1.0 + np.exp(-float(np.asarray(lambda_param).reshape(-1)[0])))
           ) * (1.0 - offset) + 0.2
    return float(np.clip(lam, 0.1, 0.9))


def prep(inputs, S=2048):
    """Host-side shard prep: returns (in_maps, bias_vec)."""
    x = np.asarray(inputs["x"], np.float32)
    T = B * S
    x2 = np.ascontiguousarray(x.reshape(T, DIM))
    xta = np.zeros((DA, T), np.float32)
    xta[:DIM] = x2.T
    xta[DIM] = 1.0

    lam = get_lambda(inputs["lambda_param"], inputs["layer_idx"])
    pw = np.asarray(inputs["proj_w"], np.float32)
    xta_mm = xta.astype(MM_NP)

    in_maps = []
    for c in range(NCORES):
        br = c // 4 + 1
        lamf = (1.0 - lam) if br == 1 else lam
        hs = slice(4 * (c % 4), 4 * (c % 4) + 4)

        def pmajor(w2d):
            # [ndt*128, m] -> [128, ndt*m] so each partition's DMA run is
            # contiguous
            ndt = w2d.shape[0] // 128
            m = w2d.shape[1]
            return np.ascontiguousarray(
                w2d.reshape(ndt, 128, m).transpose(1, 0, 2).reshape(
                    128, ndt * m)).astype(MM_NP)

        def aug_q(w, bias, scale):
            wa = np.zeros((DA, NH, HD), np.float32)
            wa[:DIM] = np.asarray(w, np.float32)[:, hs]
            wa[DIM] = np.asarray(bias, np.float32)[hs]
            return pmajor((wa * scale).reshape(DA, NH * HD))

        def plain(w):
            wa = np.asarray(w, np.float32)[:, hs]
            return pmajor(wa.reshape(DIM, NH * HD))

        wo_c = pmajor(
            (np.asarray(inputs[f"wo{br}"], np.float32)[hs] * lamf
             ).reshape(256, DIM) @ pw)
        in_maps.append({
            "xta": xta_mm,
            "wq": aug_q(inputs[f"wq{br}"], inputs[f"bq{br}"], 1.0 / np.sqrt(HD)),
            "wk": plain(inputs[f"wk{br}"]),
            "wv": plain(inputs[f"wv{br}"]),
            "wo": wo_c,
        })

    lam64 = np.float64(np.float32(lam))
    yb = np.zeros((DIM,), np.float64)
    for br, lamf in ((1, 1.0 - lam64), (2, lam64)):
        # bo and the dropped v-bias, both through wo (exact: sum_s p_s = 1)
        wof = np.asarray(inputs[f"wo{br}"], np.float64).reshape(H * HD, DIM)
        yb += lamf * (np.asarray(inputs[f"bo{br}"], np.float64)
                      + np.asarray(inputs[f"bv{br}"], np.float64).reshape(H * HD)
                      @ wof)
    bias_vec = yb @ pw.astype(np.float64) \
        + np.asarray(inputs["proj_b"], np.float64)
    return in_maps, bias_vec


_NC_CACHE = {}


def _get_nc(S=2048):
    if S not in _NC_CACHE:
        _NC_CACHE[S] = build(S)
    return _NC_CACHE[S]


def run(inputs, S=2048, trace=False):
    """Returns (full_output, exec_time_ns_or_None)."""
    from concourse import bass_utils

    nc = _get_nc(S)
    in_maps, bias_vec = prep(inputs, S)
    res = bass_utils.run_bass_kernel_spmd(
        nc, in_maps, core_ids=list(range(NCORES)), trace=trace)
    accT = np.zeros((DIM, B * S), np.float64)
    for c in range(NCORES):
        accT += res.results[c]["z"].astype(np.float64)
    out = (accT.T + bias_vec).reshape(B, S, DIM).astype(np.float32)
    return out, res.exec_time_ns


def kernel(**inputs):
    out, _ = run(inputs, S=2048, trace=False)
    return out
